# revision 5
# baseline (speedup 1.0000x reference)
"""Self-contained Trainium2 Bass kernel for nn_EncoderModel (2-layer hetero
GraphSAGE + dot-product decode) on 8 NeuronCores.

Strategy (per spec sharding hint, adapted):
  - Users are relabeled by degree (desc) and dealt round-robin to 8 cores;
    each core owns 62500 users (padded to 62976 = 123*512), split into two
    sub-tables of 31488 rows so gather indices fit int16.
  - Courses (10000, padded to 10112) are relabeled by degree; course tables
    are replicated on every core.
  - All sparse ops are dma_gather (SWDGE custom op) from padded-CSR slot
    lists precomputed on the host: per-128-node groups padded to the group
    max degree, pad slots pointing at an all-zero table row.
  - user->course aggregation is computed as per-core partials over the
    core's own edges, then AllReduce'd (1.3 MB) across the 8 cores.
  - Dense mixes run feature-major on the TensorEngine via PE transposes.
  - Decode gathers ou/oc rows per label and reduces on the DVE.

kernel(**inputs) takes the full problem inputs and returns the full [L]
fp32 output. Host-side numpy does only integer index/permutation work and
layout staging; all float math runs on the NeuronCores.
"""
import numpy as np

NU, NC, E, L = 500000, 10000, 2000000, 500000
GH = 16
NCORES = 8
P = 128
BATCH = 512
NBATCH = 123                 # user batches per core
UPC = NBATCH * BATCH         # 62976 padded users per core
SUB = UPC // 2               # 31488 rows per user sub-table (< int16 max)
ZSUB = SUB                   # zero row index in each user sub-table
NGSUB = SUB // P             # 246 user 128-groups per sub-table
NGC = 79                     # course groups
CPAD = NGC * P               # 10112 padded courses
ZCT = CPAD                   # zero row in course tables
MAXC_CALL = 64               # max chunks (of 128 idxs) per dma_gather call
RPC = NU // NCORES           # 62500 real users per core
P5SLAB = 4096                # decode idxs per gather call


def _wrap_idx(flat):
    """int16 flat idx list (len % 16 == 0) -> [128, len/16] wrapped for the
    Q7 SWDGE readers (16-partition wrap, replicated for all 8 Q7 cores)."""
    w = flat.reshape(-1, 16).T
    return np.ascontiguousarray(np.tile(w, (8, 1)).astype(np.int16))


def _split_parts(n, cap=MAXC_CALL):
    out = []
    o = 0
    while o < n:
        c = min(cap, n - o)
        out.append((o, c))
        o += c
    return out


def preprocess(inputs):
    es = np.asarray(inputs["edge_src"]).astype(np.int64)
    ed = np.asarray(inputs["edge_dst"]).astype(np.int64)
    ls = np.asarray(inputs["label_src"]).astype(np.int64)
    ld = np.asarray(inputs["label_dst"]).astype(np.int64)
    uni = np.asarray(inputs["user_node_index"]).astype(np.int64)
    cni = np.asarray(inputs["course_node_index"]).astype(np.int64)

    deg_u = np.bincount(es, minlength=NU)
    deg_c = np.bincount(ed, minlength=NC)

    order_u = np.argsort(-deg_u, kind="stable")
    rank_u = np.empty(NU, np.int64)
    rank_u[order_u] = np.arange(NU)
    core_u = rank_u % NCORES
    pos_u = rank_u // NCORES

    order_c = np.argsort(-deg_c, kind="stable")
    newc = np.empty(NC, np.int64)
    newc[order_c] = np.arange(NC)

    ue_full = np.asarray(inputs["user_embed"], np.float32)[uni]
    ux_full = np.asarray(inputs["user_x"], np.float32)
    ce_full = np.asarray(inputs["course_embed"], np.float32)[cni]
    cx_full = np.asarray(inputs["course_x"], np.float32)

    e_core = core_u[es]
    e_pos = pos_u[es]
    e_cn = newc[ed]

    # pass 1: global (cross-core) group max degrees
    Kb_cores = np.zeros((NCORES, NBATCH), np.int64)
    K2_cores = np.zeros((NCORES, NGC, 2), np.int64)
    for k in range(NCORES):
        m = e_core == k
        cnt = np.bincount(e_pos[m], minlength=UPC)
        Kb_cores[k] = cnt.reshape(NBATCH, BATCH).max(axis=1)
        sub = e_pos[m] // SUB
        for s in (0, 1):
            cnt2 = np.bincount(e_cn[m][sub == s], minlength=CPAD)
            K2_cores[k, :, s] = cnt2.reshape(NGC, P).max(axis=1)
    Kb = Kb_cores.max(axis=0)
    K2 = K2_cores.max(axis=0)

    cb_u = np.zeros(NBATCH + 1, np.int64)
    cb_u[1:] = np.cumsum(4 * Kb)
    TU = int(cb_u[-1])
    cb_c = np.zeros((NGC, 2), np.int64)
    flatk = K2.reshape(-1)
    csum = np.concatenate([[0], np.cumsum(flatk)])
    cb_c[:, 0] = csum[0:-1:2]
    cb_c[:, 1] = csum[1::2]
    TC = int(csum[-1])

    l_core = core_u[ls]
    l_pos = pos_u[ls]
    l_sub = l_pos // SUB
    l_cn = newc[ld]
    nl = np.zeros((NCORES, 2), np.int64)
    for k in range(NCORES):
        for s in (0, 1):
            nl[k, s] = np.count_nonzero((l_core == k) & (l_sub == s))
    NL = [int(np.ceil(max(nl[:, s].max(), 1) / P) * P) for s in (0, 1)]

    meta = {"Kb": Kb, "K2": K2, "cb_u": cb_u, "cb_c": cb_c,
            "TU": TU, "TC": TC, "NL": NL}

    # shared (replicated) device inputs
    ce = np.zeros((CPAD, GH), np.float32)
    cx = np.zeros((CPAD, 2), np.float32)
    ce[:NC] = ce_full[order_c]
    cx[:NC] = cx_full[order_c]
    inv_c = np.ones(CPAD, np.float32)
    inv_c[:NC] = 1.0 / np.maximum(deg_c[order_c], 1.0)
    ct_init = np.zeros((CPAD + 1, 64), np.float32)
    ct_init[:CPAD, 0:16] = ce

    out_map = np.zeros((L, 2), np.int64)

    cores = []
    for k in range(NCORES):
        old = order_u[k::NCORES]
        ue = np.zeros((UPC, GH), np.float32)
        ux = np.zeros((UPC, 5), np.float32)
        ue[:RPC] = ue_full[old]
        ux[:RPC] = ux_full[old]
        inv_u = np.ones(UPC, np.float32)
        inv_u[:RPC] = 1.0 / np.maximum(deg_u[old], 1.0)
        xu_init0 = np.zeros((SUB + 1, 64), np.float32)
        xu_init1 = np.zeros((SUB + 1, 64), np.float32)
        xu_init0[:SUB, 0:16] = ue[:SUB]
        xu_init1[:SUB, 0:16] = ue[SUB:]
        uxT = np.ascontiguousarray(ux.T)

        m = e_core == k
        ep = e_pos[m]
        ecn = e_cn[m]

        o = np.argsort(ep, kind="stable")
        ep_s, ecn_s = ep[o], ecn[o]
        cnt = np.bincount(ep_s, minlength=UPC)
        off = np.concatenate([[0], np.cumsum(cnt)])
        jr = np.arange(len(ep_s)) - np.repeat(off[:-1], cnt)
        b = ep_s // BATCH
        q = (ep_s % BATCH) // P
        pp = ep_s % P
        chunk = cb_u[b] + jr * 4 + q
        slots_u = np.full(TU * P, ZCT, np.int16)
        slots_u[chunk * P + pp] = ecn_s.astype(np.int16)

        sub = ep // SUB
        loc = ep % SUB
        slots_c = np.full(TC * P, ZSUB, np.int16)
        for s in (0, 1):
            m2 = sub == s
            ec2, lc2 = ecn[m2], loc[m2]
            o2 = np.argsort(ec2, kind="stable")
            ec2, lc2 = ec2[o2], lc2[o2]
            cnt2 = np.bincount(ec2, minlength=CPAD)
            off2 = np.concatenate([[0], np.cumsum(cnt2)])
            jr2 = np.arange(len(ec2)) - np.repeat(off2[:-1], cnt2)
            chunk2 = cb_c[ec2 // P, s] + jr2
            slots_c[chunk2 * P + (ec2 % P)] = lc2.astype(np.int16)

        lm = np.nonzero(l_core == k)[0]
        lab_ou = [np.full(NL[s], ZSUB, np.int16) for s in (0, 1)]
        lab_oc = [np.full(NL[s], ZCT, np.int16) for s in (0, 1)]
        for s in (0, 1):
            sel = lm[l_sub[lm] == s]
            n = len(sel)
            lab_ou[s][:n] = (l_pos[sel] % SUB).astype(np.int16)
            lab_oc[s][:n] = l_cn[sel].astype(np.int16)
            base = 0 if s == 0 else NL[0]
            out_map[sel, 0] = k
            out_map[sel, 1] = base + np.arange(n)

        cores.append({
            "uxT": uxT, "xu_init0": xu_init0, "xu_init1": xu_init1,
            "inv_u": inv_u,
            "u_idx": _wrap_idx(slots_u), "c_idx": _wrap_idx(slots_c),
            "lab_ou": _wrap_idx(np.concatenate(lab_ou)),
            "lab_oc": _wrap_idx(np.concatenate(lab_oc)),
        })

    def T(x):
        return np.ascontiguousarray(np.asarray(x, np.float32).T)

    shared = {
        "cxT": np.ascontiguousarray(cx.T), "ct_init": ct_init, "inv_c": inv_c,
        "WuT": T(inputs["Wu"]),
        "bu": np.asarray(inputs["bu"], np.float32).reshape(GH, 1),
        "WcT": T(inputs["Wc"]),
        "bc": np.asarray(inputs["bc"], np.float32).reshape(GH, 1),
        "Wh": np.ascontiguousarray(np.concatenate(
            [T(inputs["c1r_Wl"]), np.zeros((32, 32), np.float32),
             T(inputs["c1r_Wr"])], axis=0)),
        "bh": np.asarray(inputs["c1r_bl"], np.float32).reshape(32, 1),
        "Woa": np.ascontiguousarray(np.concatenate(
            [np.zeros((32, 16), np.float32), T(inputs["c2r_Wl"]),
             np.zeros((32, 16), np.float32)], axis=0)),
        "Wob": T(inputs["c2r_Wr"]),
        "bo": np.asarray(inputs["c2r_bl"], np.float32).reshape(16, 1),
        "Whc": np.ascontiguousarray(np.concatenate(
            [T(inputs["c1e_Wl"]), T(inputs["c1e_Wr"])], axis=0)),
        "bhc": np.asarray(inputs["c1e_bl"], np.float32).reshape(32, 1),
        "Woc": np.ascontiguousarray(np.concatenate(
            [T(inputs["c2e_Wl"]), T(inputs["c2e_Wr"])], axis=0)),
        "boc": np.asarray(inputs["c2e_bl"], np.float32).reshape(16, 1),
    }
    meta["out_map"] = out_map
    return meta, shared, cores


def build(meta):
    """Build + compile the (SPMD, identical across cores) Bass program."""
    from concourse import bass, mybir, tile, bacc
    from concourse.masks import make_identity

    Kb = meta["Kb"]
    K2 = meta["K2"]
    cb_u = meta["cb_u"]
    cb_c = meta["cb_c"]
    TU, TC, NL = meta["TU"], meta["TC"], meta["NL"]
    NLTOT = NL[0] + NL[1]
    f32 = mybir.dt.float32
    i16 = mybir.dt.int16

    nc = bacc.Bacc("TRN2", target_bir_lowering=False, debug=False,
                   num_devices=NCORES)

    def din(name, shape, dt=f32):
        return nc.dram_tensor(name, shape, dt, kind="ExternalInput")

    uxT = din("uxT", [5, UPC])
    xu_init = [din("xu_init0", [SUB + 1, 64]), din("xu_init1", [SUB + 1, 64])]
    inv_u = din("inv_u", [UPC])
    u_idx = din("u_idx", [P, TU * 8], i16)
    c_idx = din("c_idx", [P, TC * 8], i16)
    lab_ou = din("lab_ou", [P, NLTOT // 16], i16)
    lab_oc = din("lab_oc", [P, NLTOT // 16], i16)
    cxT = din("cxT", [2, CPAD])
    ct_init = din("ct_init", [CPAD + 1, 64])
    inv_c = din("inv_c", [CPAD])
    W = {n: din(n, s) for n, s in [
        ("WuT", [5, 16]), ("bu", [16, 1]), ("WcT", [2, 16]), ("bc", [16, 1]),
        ("Wh", [96, 32]), ("bh", [32, 1]),
        ("Woa", [96, 16]), ("Wob", [32, 16]), ("bo", [16, 1]),
        ("Whc", [64, 32]), ("bhc", [32, 1]),
        ("Woc", [64, 16]), ("boc", [16, 1])]}
    out = nc.dram_tensor("out", [NLTOT], f32, kind="ExternalOutput")

    AP = bass.AP

    with tile.TileContext(nc) as tc:
        with (
            tc.tile_pool(name="const", bufs=1) as cs,
            tc.tile_pool(name="sbuf", bufs=2) as sb,
            tc.tile_pool(name="psum", bufs=2, space="PSUM") as ps,
            tc.tile_pool(name="dram", bufs=1, space="DRAM") as dr,
        ):
            # ---------- constants
            id128 = cs.tile([P, P], f32)
            make_identity(nc, id128[:])
            id32 = cs.tile([32, 32], f32)
            make_identity(nc, id32[:])
            wt = {}
            for n, h in W.items():
                t = cs.tile(list(h.shape), f32, name=f"w_{n}")
                nc.sync.dma_start(out=t[:], in_=h[:, :])
                wt[n] = t

            # ---------- DRAM scratch tables
            xu = [dr.tile([SUB + 1, 64], f32, name="xu0"),
                  dr.tile([SUB + 1, 64], f32, name="xu1")]
            hu = [dr.tile([SUB + 1, 64], f32, name="hu0"),
                  dr.tile([SUB + 1, 64], f32, name="hu1")]
            ou = [dr.tile([SUB + 1, 64], f32, name="ou0"),
                  dr.tile([SUB + 1, 64], f32, name="ou1")]
            ct = dr.tile([CPAD + 1, 64], f32, name="ct")
            oc = dr.tile([CPAD + 1, 64], f32, name="oc")
            part1 = dr.tile([CPAD, 32], f32, name="part1")
            ar1 = dr.tile([CPAD, 32], f32, name="ar1", addr_space="Shared")
            part2 = dr.tile([CPAD, 32], f32, name="part2")
            ar2 = dr.tile([CPAD, 32], f32, name="ar2", addr_space="Shared")

            # table init copies + zero rows (row-chunked: one flat DMA
            # would overflow the 16-bit per-descriptor element field)
            for r0 in range(0, CPAD + 1, 8192):
                r1 = min(r0 + 8192, CPAD + 1)
                nc.sync.dma_start(out=ct[r0:r1, :], in_=ct_init[r0:r1, :])
            for s in (0, 1):
                for r0 in range(0, SUB + 1, 8192):
                    r1 = min(r0 + 8192, SUB + 1)
                    nc.sync.dma_start(out=xu[s][r0:r1, :],
                                      in_=xu_init[s][r0:r1, :])
            zrow = cs.tile([1, 64], f32)
            nc.gpsimd.memset(zrow[:], 0.0)
            for s in (0, 1):
                nc.sync.dma_start(out=hu[s][SUB:SUB + 1, :], in_=zrow[:])
                nc.sync.dma_start(out=ou[s][SUB:SUB + 1, :], in_=zrow[:])
            nc.sync.dma_start(out=oc[CPAD:CPAD + 1, :], in_=zrow[:])

            def proj_pass(src_h, w_lhs, w_bias, n_total, dst_rows):
                """Project [k_in, n_total] feature-major input through
                lhsT [k_in, 16], add bias, transpose, write 16-wide rows
                via dst_rows(group_idx) -> (table_ap, row0, col0)."""
                k_in = src_h.shape[0]
                for base in range(0, n_total, BATCH):
                    n = min(BATCH, n_total - base)
                    rhs = sb.tile([k_in, BATCH], f32, tag="proj_rhs")
                    nc.sync.dma_start(out=rhs[:, :n],
                                      in_=src_h[:, base:base + n])
                    pp = ps.tile([16, BATCH], f32, tag="psA")
                    nc.tensor.matmul(out=pp[:, :n], lhsT=w_lhs[:],
                                     rhs=rhs[:, :n], start=True, stop=True)
                    sp = sb.tile([32, BATCH], f32, tag="proj_sb")
                    nc.scalar.activation(
                        out=sp[0:16, :n], in_=pp[:, :n],
                        func=mybir.ActivationFunctionType.Identity,
                        bias=w_bias[:])
                    for t in range(0, n, P):
                        tr = ps.tile([P, 32], f32, tag="tr")
                        nc.tensor.transpose(out=tr[:], in_=sp[:, t:t + P],
                                            identity=id32[:])
                        trs = sb.tile([P, 32], f32, tag="trs")
                        nc.vector.tensor_copy(out=trs[:], in_=tr[:])
                        tab, r0, c0 = dst_rows((base + t) // P)
                        nc.sync.dma_start(
                            out=tab[r0:r0 + P, c0:c0 + 16],
                            in_=trs[:, 0:16])

            # ---------- P0: course projection into ct[:,16:32]
            proj_pass(cxT, wt["WcT"], wt["bc"], CPAD,
                      lambda g: (ct, g * P, 16))
            # ---------- P1: user projection into xu[s][:,16:32]
            proj_pass(uxT, wt["WuT"], wt["bu"], UPC,
                      lambda g: (xu[g // NGSUB], (g % NGSUB) * P, 16))

            # ---------- course-side aggregation pass (used for P2 and P4)
            def course_agg(tables, dst):
                for gc in range(NGC):
                    k0, k1 = int(K2[gc, 0]), int(K2[gc, 1])
                    nch = k0 + k1
                    if nch == 0:
                        continue
                    gt = sb.tile([P, nch * 64], f32, tag="cslab",
                                 name=f"gt_{gc}")
                    idx = sb.tile([P, nch * 8], i16, tag="cidx",
                                  name=f"cix_{gc}")
                    col0 = int(cb_c[gc, 0]) * 8
                    nc.sync.dma_start(out=idx[:],
                                      in_=c_idx[:, col0:col0 + nch * 8])
                    for s, kk in ((0, k0), (1, k1)):
                        base_ch = 0 if s == 0 else k0
                        for (po, pn) in _split_parts(kk):
                            o = base_ch + po
                            nc.gpsimd.dma_gather(
                                out_ap=gt[:, o * 64:(o + pn) * 64].rearrange(
                                    "p (c e) -> p c e", e=64),
                                in_ap=tables[s][:, :],
                                idxs_ap=idx[:, o * 8:(o + pn) * 8],
                                num_idxs=pn * P,
                                num_idxs_reg=pn * P,
                                elem_size=64,
                                single_packet=False,
                            )
                    acc = sb.tile([P, 32], f32, tag="cacc")
                    nc.vector.reduce_sum(
                        out=acc[:],
                        in_=gt[:].rearrange("p (c e) -> p e c",
                                            e=64)[:, 0:32, :],
                        axis=mybir.AxisListType.X)
                    nc.sync.dma_start(out=dst[gc * P:(gc + 1) * P, :],
                                      in_=acc[:])

            # ---------- P2: agg1_c partials from xu; AllReduce
            course_agg(xu, part1)
            nc.gpsimd.collective_compute(
                "AllReduce", mybir.AluOpType.add,
                replica_groups=[list(range(NCORES))],
                ins=[part1[:, :]], outs=[ar1[:, :]])

            # ---------- P2.5: hc = relu(Whc @ [agg*inv ; xc]) into ct[:,32:64]
            def course_mix(ar, src_cols, w_lhs, w_bias, relu, dst,
                           dst_cols):
                for gc in range(NGC):
                    r0 = gc * P
                    mc = sb.tile([P, 64], f32, tag="mc")
                    nc.sync.dma_start(out=mc[:, 0:32],
                                      in_=ar[r0:r0 + P, :])
                    nc.sync.dma_start(
                        out=mc[:, 32:64],
                        in_=ct[r0:r0 + P, src_cols:src_cols + 32])
                    ivt = sb.tile([P, 1], f32, tag="ivc")
                    nc.sync.dma_start(out=ivt[:],
                                      in_=AP(inv_c, r0, [[1, P]]))
                    nc.vector.tensor_tensor(
                        out=mc[:, 0:32], in0=mc[:, 0:32],
                        in1=ivt[:].to_broadcast([P, 32]),
                        op=mybir.AluOpType.mult)
                    tp = ps.tile([64, P], f32, tag="psA")
                    nc.tensor.transpose(out=tp[:], in_=mc[:],
                                        identity=id128[:])
                    ts = sb.tile([64, P], f32, tag="cts")
                    nc.vector.tensor_copy(out=ts[:], in_=tp[:])
                    nout = w_lhs.shape[1]
                    pm = ps.tile([nout, P], f32, tag="psB")
                    nc.tensor.matmul(out=pm[:], lhsT=w_lhs[:],
                                     rhs=ts[:], start=True, stop=True)
                    hs = sb.tile([32, P], f32, tag="chs")
                    nc.scalar.activation(
                        out=hs[0:nout, :], in_=pm[:],
                        func=(mybir.ActivationFunctionType.Relu if relu
                              else mybir.ActivationFunctionType.Identity),
                        bias=w_bias[:])
                    tr = ps.tile([P, 32], f32, tag="tr")
                    nc.tensor.transpose(out=tr[:], in_=hs[:],
                                        identity=id32[:])
                    trs = sb.tile([P, 32], f32, tag="trs")
                    nc.vector.tensor_copy(out=trs[:], in_=tr[:])
                    nc.sync.dma_start(
                        out=dst[r0:r0 + P, dst_cols:dst_cols + (
                            16 if nout == 16 else 32)],
                        in_=trs[:, 0:(16 if nout == 16 else 32)])

            course_mix(ar1, 0, wt["Whc"], wt["bhc"], True, ct, 32)

            # ---------- P3: user batches
            for b in range(NBATCH):
                K = int(Kb[b])
                nch = 4 * K
                mt = sb.tile([P, 4 * 96], f32, tag="m", name=f"m_{b}")
                if K > 0:
                    gt = sb.tile([P, nch * 64], f32, tag="uslab",
                                 name=f"ug_{b}")
                    idx = sb.tile([P, nch * 8], i16, tag="uidx",
                                  name=f"uix_{b}")
                    col0 = int(cb_u[b]) * 8
                    nc.sync.dma_start(out=idx[:],
                                      in_=u_idx[:, col0:col0 + nch * 8])
                    for (jo, jn) in _split_parts(K, 16):
                        o = jo * 4
                        pn = jn * 4
                        nc.gpsimd.dma_gather(
                            out_ap=gt[:, o * 64:(o + pn) * 64].rearrange(
                                "p (c e) -> p c e", e=64),
                            in_ap=ct[:, :],
                            idxs_ap=idx[:, o * 8:(o + pn) * 8],
                            num_idxs=pn * P,
                            num_idxs_reg=pn * P,
                            elem_size=64,
                            single_packet=False,
                        )
                    for q in range(4):
                        nc.vector.reduce_sum(
                            out=mt[:, q * 96:q * 96 + 64],
                            in_=gt[:].rearrange(
                                "p (j q e) -> p q e j", q=4,
                                e=64)[:, q, :, :],
                            axis=mybir.AxisListType.X)
                else:
                    nc.gpsimd.memset(mt[:], 0.0)
                iv4 = sb.tile([P, 4], f32, tag="iv4")
                nc.sync.dma_start(out=iv4[:],
                                  in_=AP(inv_u, b * BATCH, [[1, P], [P, 4]]))
                nc.vector.tensor_tensor(
                    out=mt[:].rearrange("p (q e) -> p q e", e=96)[:, :, 0:64],
                    in0=mt[:].rearrange("p (q e) -> p q e", e=96)[:, :, 0:64],
                    in1=iv4[:].to_broadcast([P, 4, 64]),
                    op=mybir.AluOpType.mult)
                for t in range(4):
                    g = b * 4 + t
                    s, r0 = g // NGSUB, (g % NGSUB) * P
                    nc.sync.dma_start(
                        out=mt[:, t * 96 + 64:t * 96 + 96],
                        in_=xu[s][r0:r0 + P, 0:32])
                tp = ps.tile([96, BATCH], f32, tag="psA")
                for q in range(4):
                    nc.tensor.transpose(
                        out=tp[:, q * P:(q + 1) * P],
                        in_=mt[:, q * 96:(q + 1) * 96],
                        identity=id128[:])
                ts = sb.tile([96, BATCH], f32, tag="uts")
                nc.vector.tensor_copy(out=ts[:], in_=tp[:])
                ph = ps.tile([32, BATCH], f32, tag="psB")
                nc.tensor.matmul(out=ph[:], lhsT=wt["Wh"][:], rhs=ts[:],
                                 start=True, stop=True)
                hs = sb.tile([32, BATCH], f32, tag="uhs")
                nc.scalar.activation(out=hs[:], in_=ph[:],
                                     func=mybir.ActivationFunctionType.Relu,
                                     bias=wt["bh"][:])
                po = ps.tile([16, BATCH], f32, tag="psB")
                nc.tensor.matmul(out=po[:], lhsT=wt["Woa"][:],
                                 rhs=ts[:], start=True, stop=False)
                nc.tensor.matmul(out=po[:], lhsT=wt["Wob"][:], rhs=hs[:],
                                 start=False, stop=True)
                os_ = sb.tile([32, BATCH], f32, tag="uos")
                nc.scalar.activation(
                    out=os_[0:16, :], in_=po[:],
                    func=mybir.ActivationFunctionType.Identity,
                    bias=wt["bo"][:])
                for t in range(4):
                    g = b * 4 + t
                    s, r0 = g // NGSUB, (g % NGSUB) * P
                    tr = ps.tile([P, 32], f32, tag="tr")
                    nc.tensor.transpose(out=tr[:], in_=hs[:, t * P:(t + 1) * P],
                                        identity=id32[:])
                    trs = sb.tile([P, 32], f32, tag="trs")
                    nc.vector.tensor_copy(out=trs[:], in_=tr[:])
                    nc.sync.dma_start(out=hu[s][r0:r0 + P, 0:32],
                                      in_=trs[:])
                    tr2 = ps.tile([P, 32], f32, tag="tr")
                    nc.tensor.transpose(out=tr2[:],
                                        in_=os_[:, t * P:(t + 1) * P],
                                        identity=id32[:])
                    trs2 = sb.tile([P, 32], f32, tag="trs")
                    nc.vector.tensor_copy(out=trs2[:], in_=tr2[:])
                    nc.sync.dma_start(out=ou[s][r0:r0 + P, 0:16],
                                      in_=trs2[:, 0:16])

            # ---------- P4: agg2_c partials from hu; AllReduce
            course_agg(hu, part2)
            nc.gpsimd.collective_compute(
                "AllReduce", mybir.AluOpType.add,
                replica_groups=[list(range(NCORES))],
                ins=[part2[:, :]], outs=[ar2[:, :]])

            # ---------- P4.5: oc
            course_mix(ar2, 32, wt["Woc"], wt["boc"], False, oc, 0)

            # ---------- P5: decode
            for s in (0, 1):
                gbase = 0 if s == 0 else NL[0]
                for sl0 in range(0, NL[s], P5SLAB):
                    n = min(P5SLAB, NL[s] - sl0)
                    C = n // P
                    iu = sb.tile([P, n // 16], i16, tag="liu")
                    ic = sb.tile([P, n // 16], i16, tag="lic")
                    c0 = (gbase + sl0) // 16
                    nc.sync.dma_start(out=iu[:],
                                      in_=lab_ou[:, c0:c0 + n // 16])
                    nc.sync.dma_start(out=ic[:],
                                      in_=lab_oc[:, c0:c0 + n // 16])
                    gu = sb.tile([P, C * 64], f32, tag="lgu")
                    gc2 = sb.tile([P, C * 64], f32, tag="lgc")
                    for (po, pn) in _split_parts(C):
                        for gtile, tab, it in ((gu, ou[s], iu),
                                               (gc2, oc, ic)):
                            nc.gpsimd.dma_gather(
                                out_ap=gtile[:, po * 64:(po + pn) * 64]
                                .rearrange("p (c e) -> p c e", e=64),
                                in_ap=tab[:, :],
                                idxs_ap=it[:, po * 8:(po + pn) * 8],
                                num_idxs=pn * P,
                                num_idxs_reg=pn * P,
                                elem_size=64,
                                single_packet=False,
                            )
                    pr = sb.tile([P, C * 16], f32, tag="lpr")
                    nc.vector.tensor_tensor(
                        out=pr[:].rearrange("p (c e) -> p c e", e=16),
                        in0=gu[:].rearrange("p (c e) -> p c e",
                                            e=64)[:, :, 0:16],
                        in1=gc2[:].rearrange("p (c e) -> p c e",
                                             e=64)[:, :, 0:16],
                        op=mybir.AluOpType.mult)
                    rd = sb.tile([P, C], f32, tag="lrd")
                    nc.vector.reduce_sum(
                        out=rd[:],
                        in_=pr[:].rearrange("p (c e) -> p c e", e=16),
                        axis=mybir.AxisListType.X)
                    nc.sync.dma_start(
                        out=AP(out, gbase + sl0, [[1, P], [P, C]]),
                        in_=rd[:])

    nc.compile()
    return nc


_cache = {}


def kernel(**inputs):
    from concourse.bass_utils import run_bass_kernel_spmd

    meta, shared, cores = preprocess(inputs)
    key = (tuple(meta["Kb"].tolist()), tuple(map(tuple, meta["K2"].tolist())),
           tuple(meta["NL"]))
    if key not in _cache:
        _cache[key] = build(meta)
    nc = _cache[key]

    in_maps = []
    for k in range(NCORES):
        m = dict(shared)
        m.update(cores[k])
        m.pop("slots_u", None)
        m.pop("slots_c", None)
        in_maps.append({
            "uxT": cores[k]["uxT"], "xu_init0": cores[k]["xu_init0"],
            "xu_init1": cores[k]["xu_init1"], "inv_u": cores[k]["inv_u"],
            "u_idx": cores[k]["u_idx"], "c_idx": cores[k]["c_idx"],
            "lab_ou": cores[k]["lab_ou"], "lab_oc": cores[k]["lab_oc"],
            **{n: shared[n] for n in
               ("cxT", "ct_init", "inv_c", "WuT", "bu", "WcT", "bc", "Wh",
                "bh", "Woa", "Wob", "bo", "Whc", "bhc", "Woc", "boc")},
        })
    import sys as _sys
    _mod = _sys.modules[kernel.__module__]
    res = run_bass_kernel_spmd(nc, in_maps, core_ids=list(range(NCORES)),
                               trace=getattr(_mod, "TRACE", False),
                               trace_cores=getattr(_mod, "TRACE_CORES", None))
    kernel.last_results = res

    om = meta["out_map"]
    outs = np.stack([res.results[k]["out"] for k in range(NCORES)])
    return outs[om[:, 0], om[:, 1]].astype(np.float32)
